# revision 24
# baseline (speedup 1.0000x reference)
"""Multi-head self-attention kernel for 8 Trainium2 NeuronCores.

Sharding: core c = (b, g) with b = batch index (4), g = head-group (2).
Each core computes attention for one batch element and 8 of the 16 heads,
including its slice of the QKV projections and a partial out-projection
(Y_partial = O_heads @ Wo[rows of its heads]).  The host sums the two
head-group partials per batch and transposes (the device produces Y^T).

On-device layout is fully "transposed": x^T [D, S] in, Q^T/K^T [dk, S],
scores S^T = K_h Q_h^T [k, q] (softmax along partitions via a ones-column
appended to V: the PV matmul O^T_aug = [V|1]^T P^T yields the softmax
denominator in its last row), output Y^T [D, S].

Phase 1 computes K^T and V (resident); phase 2 projects Q^T per
(pair, q-block) just-in-time so the PE's projection work overlaps the
activation engine's exp stream, which is the throughput bottleneck.
"""

import sys

sys.path.insert(0, "/opt/trn_rl_repo")

from contextlib import ExitStack

import numpy as np

import concourse.bass as bass
import concourse.tile as tile
from concourse import bacc, mybir
from concourse.bass_utils import run_bass_kernel_spmd

F32 = mybir.dt.float32
F32R = mybir.dt.float32r
BF16 = mybir.dt.bfloat16
P = 128  # SBUF partitions

D_MODEL = 1024
NHEAD = 16
DK = D_MODEL // NHEAD  # 64
BATCH = 4
SEQ = 2048
N_CORES = 8
HL = NHEAD // 2  # heads per core (head-group of 8)


def build_bass(D=D_MODEL, S=SEQ, HLOC=HL, QB=512, repeat=1, p2loop=1):
    """Build the per-core Bass program (same program on all 8 cores)."""
    DC = D // P           # d_model chunks (contraction for projections)
    KC = S // P           # key chunks
    NQB = S // QB         # q blocks
    NPAIR = HLOC // 2     # head pairs
    HD = HLOC * DK        # local head dim total (512)
    VW = DK + 1           # V columns per head incl. ones column
    NOC = D // P          # out-dim chunks
    EXP_SCALE = 1.0 / np.sqrt(DK)
    cfg = dict(D=D, S=S, HLOC=HLOC, QB=QB, DC=DC, KC=KC, NQB=NQB,
               NPAIR=NPAIR, HD=HD, VW=VW, NOC=NOC, EXP_SCALE=EXP_SCALE)

    nc = bacc.Bacc("TRN2", target_bir_lowering=False, debug=False,
                   num_devices=N_CORES)

    xT = nc.dram_tensor("xT", [D, S], F32, kind="ExternalInput")
    Wq = nc.dram_tensor("Wq", [D, HD], F32, kind="ExternalInput")
    Wk = nc.dram_tensor("Wk", [D, HD], F32, kind="ExternalInput")
    Wv = nc.dram_tensor("Wv", [D, HD], F32, kind="ExternalInput")
    Wo = nc.dram_tensor("Wo", [HD, D], F32, kind="ExternalInput")
    bq_t = nc.dram_tensor("bq_t", [P, NPAIR], F32, kind="ExternalInput")
    bk_t = nc.dram_tensor("bk_t", [P, NPAIR], F32, kind="ExternalInput")
    bv_bc = nc.dram_tensor("bv_bc", [P, HD], F32, kind="ExternalInput")
    bo_t = nc.dram_tensor("bo_t", [P, NOC], F32, kind="ExternalInput")
    YT = nc.dram_tensor("YT", [D, S], F32, kind="ExternalOutput")
    dram = dict(xT=xT, Wq=Wq, Wk=Wk, Wv=Wv, Wo=Wo, bq_t=bq_t, bk_t=bk_t,
                bv_bc=bv_bc, bo_t=bo_t, YT=YT)

    with tile.TileContext(nc) as tc, ExitStack() as ctx:
        consts = ctx.enter_context(tc.tile_pool(name="consts", bufs=1))
        ktv = ctx.enter_context(tc.tile_pool(name="ktv", bufs=1))
        wper = ctx.enter_context(tc.tile_pool(name="wper", bufs=1))
        xres = ctx.enter_context(tc.tile_pool(name="xres", bufs=1))
        ps_a = ctx.enter_context(tc.tile_pool(name="ps_a", bufs=2, space="PSUM"))
        ps_b = ctx.enter_context(tc.tile_pool(name="ps_b", bufs=2, space="PSUM"))
        ps_acc = ctx.enter_context(tc.tile_pool(name="ps_acc", bufs=2,
                                                space="PSUM"))

        # ---- constants ----
        bq_sb = consts.tile([P, NPAIR], F32, tag="bq")
        bk_sb = consts.tile([P, NPAIR], F32, tag="bk")
        bv_sb = consts.tile([P, HD], F32, tag="bv")
        bo_sb = consts.tile([P, NOC], F32, tag="bo")
        nc.sync.dma_start(bq_sb[:], bq_t.ap())
        nc.sync.dma_start(bk_sb[:], bk_t.ap())
        nc.sync.dma_start(bv_sb[:], bv_bc.ap())
        nc.sync.dma_start(bo_sb[:], bo_t.ap())

        # warm the ACT exp table early
        warm = consts.tile([1, 2], F32, tag="warm")
        nc.gpsimd.memset(warm[0:1, 0:1], 0.0)
        nc.scalar.activation(warm[0:1, 1:2], warm[0:1, 0:1],
                             mybir.ActivationFunctionType.Exp)

        ones_sb = consts.tile([P, HLOC], F32, tag="ones")
        nc.vector.memset(ones_sb[:], 1.0)

        sbs = dict(bq=bq_sb, bk=bk_sb, bv=bv_sb, bo=bo_sb, ones=ones_sb)
        pools = dict(consts=consts, ktv=ktv, wper=wper, xres=xres,
                     ps_a=ps_a, ps_b=ps_b, ps_acc=ps_acc)

        for _rep in range(repeat):
            emit_body(nc, tc, cfg, dram, sbs, pools, p2loop=p2loop)

    nc.compile()
    return nc


def emit_body(nc, tc, cfg, dram, sbs, pools, p2loop=1):
    D, S, HLOC, QB = cfg["D"], cfg["S"], cfg["HLOC"], cfg["QB"]
    DC, KC, NQB, NPAIR = cfg["DC"], cfg["KC"], cfg["NQB"], cfg["NPAIR"]
    HD, VW, NOC, EXP_SCALE = cfg["HD"], cfg["VW"], cfg["NOC"], cfg["EXP_SCALE"]
    ktv, wper, xres = pools["ktv"], pools["wper"], pools["xres"]
    ps_a, ps_b, ps_acc = pools["ps_a"], pools["ps_b"], pools["ps_acc"]
    bq_sb, bk_sb, bv_sb = sbs["bq"], sbs["bk"], sbs["bv"]
    bo_sb, ones_sb = sbs["bo"], sbs["ones"]

    xt_dram3 = dram["xT"].ap().bitcast(F32R).rearrange("(c p) s -> p c s", p=P)
    yt_dram3 = dram["YT"].ap().rearrange("(n p) s -> p n s", p=P)

    # resident tensors
    kt_tiles = [ktv.tile([P, S], F32R, tag=f"kt{p_}", name=f"kt{p_}")
                for p_ in range(NPAIR)]
    v_tiles = [ktv.tile([P, HLOC * VW], F32R, tag=f"v{k}", name=f"v{k}")
               for k in range(KC)]
    xt = xres.tile([P, DC * S], F32R, tag="xt", name="xt")
    xt3 = xt[:].rearrange("p (c s) -> p c s", c=DC)

    def wslice(wt, c, lo, hi):
        return wt[:, c * HD + lo: c * HD + hi]

    def load_w(pool, name, d, cols):
        t = pool.tile([P, DC * cols], F32R, tag=name, name=name)
        nc.sync.dma_start(
            t[:].rearrange("p (c n) -> p c n", c=DC),
            d.ap().bitcast(F32R).rearrange("(c p) n -> p c n", p=P))
        return t

    # ---- phase 1: K^T and V (wk/wv scoped to this phase) ----
    with tc.tile_pool(name="wkv", bufs=1) as wkv:
        # DMA order = approximate arrival order: first window + wk first.
        nc.sync.dma_start(xt3[:, :, bass.ts(0, QB)],
                          xt_dram3[:, :, bass.ts(0, QB)])
        wk_sb = load_w(wkv, "wk", dram["Wk"], HD)
        wv_sb = load_w(wkv, "wv", dram["Wv"], HD)
        for w in range(1, NQB):
            sl = bass.ts(w, QB)
            nc.sync.dma_start(xt3[:, :, sl], xt_dram3[:, :, sl])
        wq_sb = load_w(wper, "wq", dram["Wq"], HD)
        wo_sb = wper.tile([P, NPAIR * D], F32R, tag="wo", name="wo")
        nc.sync.dma_start(
            wo_sb[:].rearrange("p (r n) -> p r n", r=NPAIR),
            dram["Wo"].ap().bitcast(F32R).rearrange("(r p) n -> p r n", p=P))

        for w in range(NQB):
            sl = bass.ts(w, QB)
            for pr in range(NPAIR):
                kps = ps_b.tile([P, QB], F32, tag="sp", name="kps")
                for c in range(DC):
                    nc.tensor.matmul(kps[:],
                                     wslice(wk_sb, c, pr * P, (pr + 1) * P),
                                     xt3[:, c, sl],
                                     start=(c == 0), stop=(c == DC - 1))
                nc.vector.tensor_scalar_add(kt_tiles[pr][:, sl], kps[:],
                                            bk_sb[:, pr:pr + 1])
            for s4 in range(QB // P):
                k = w * (QB // P) + s4
                vps = ps_b.tile([P, HD], F32, tag="sp", name="vps")
                for c in range(DC):
                    nc.tensor.matmul(vps[:],
                                     xt3[:, c, bass.ts(k, P)],
                                     wslice(wv_sb, c, 0, HD),
                                     start=(c == 0), stop=(c == DC - 1))
                v3 = v_tiles[k][:].rearrange("p (h v) -> p h v", h=HLOC)
                nc.vector.tensor_add(v3[:, :, 0:DK],
                                     vps[:].rearrange("p (h d) -> p h d", h=HLOC),
                                     bv_sb[:].rearrange("p (h d) -> p h d", h=HLOC))
                nc.vector.tensor_copy(v3[:, :, DK:VW], ones_sb[:].unsqueeze(2))

    # ---- phase 2: Q^T just-in-time + attention + out-projection ----
    with tc.tile_pool(name="qtp", bufs=3) as qtp, \
         tc.tile_pool(name="pexp", bufs=3) as pexp, \
         tc.tile_pool(name="otp", bufs=NPAIR + 1) as otp, \
         tc.tile_pool(name="misc", bufs=2) as misc:

        def qt_proj(qb, pr):
            qsl = bass.ts(qb, QB)
            qps = ps_b.tile([P, QB], F32, tag="sp", name="qps")
            for c in range(DC):
                nc.tensor.matmul(qps[:],
                                 wslice(wq_sb, c, pr * P, (pr + 1) * P),
                                 xt3[:, c, qsl],
                                 start=(c == 0), stop=(c == DC - 1))
            qt = qtp.tile([P, QB], F32R, tag="qt", name="qt")
            nc.vector.tensor_scalar_add(qt[:], qps[:], bq_sb[:, pr:pr + 1])
            return qt

        from contextlib import nullcontext
        loop_cm = tc.For_i(0, p2loop, 1) if p2loop > 1 else nullcontext()
        with loop_cm:
            _phase2(nc, tc, cfg, qt_proj, kt_tiles, v_tiles, wo_sb,
                    bo_sb, ones_sb, ps_a, ps_b, ps_acc, pexp, otp, misc,
                    yt_dram3)


def _phase2(nc, tc, cfg, qt_proj, kt_tiles, v_tiles, wo_sb, bo_sb, ones_sb,
            ps_a, ps_b, ps_acc, pexp, otp, misc, yt_dram3):
        D, S, HLOC, QB = cfg["D"], cfg["S"], cfg["HLOC"], cfg["QB"]
        DC, KC, NQB, NPAIR = cfg["DC"], cfg["KC"], cfg["NQB"], cfg["NPAIR"]
        HD, VW, NOC = cfg["HD"], cfg["VW"], cfg["NOC"]
        EXP_SCALE = cfg["EXP_SCALE"]
        P = 128
        DK = 64
        qt_next = qt_proj(0, 0)
        for qb in range(NQB):
            ot_tiles = []
            for pr in range(NPAIR):
                qt = qt_next
                kt = kt_tiles[pr]
                # pre-project the next (pair, qblock)'s Q^T so it's ready
                # when the PE gets there; keeps the exp stream fed.
                nxt = (qb * NPAIR + pr + 1)
                if nxt < NQB * NPAIR:
                    qt_next = qt_proj(nxt // NPAIR, nxt % NPAIR)

                oa = ps_acc.tile([VW, QB], F32, tag="acc", name="oa")
                ob = ps_acc.tile([VW, QB], F32, tag="acc", name="ob")
                for kc in range(KC):
                    ksl = bass.ts(kc, P)
                    sa = ps_a.tile([P, QB], F32, tag="sa", name="sa")
                    sb = ps_a.tile([P, QB], F32, tag="sb", name="sb")
                    nc.tensor.matmul(sa[:], kt[0:DK, ksl], qt[0:DK, :],
                                     start=True, stop=True)
                    nc.tensor.matmul(sb[:], kt[DK:P, ksl],
                                     qt[DK:P, :], start=True, stop=True)
                    ea = pexp.tile([P, QB], F32R, tag="ea", name="ea")
                    eb = pexp.tile([P, QB], F32R, tag="eb", name="eb")
                    nc.scalar.activation(ea[:], sa[:],
                                         mybir.ActivationFunctionType.Exp,
                                         scale=float(EXP_SCALE))
                    nc.scalar.activation(eb[:], sb[:],
                                         mybir.ActivationFunctionType.Exp,
                                         scale=float(EXP_SCALE))
                    vt = v_tiles[kc]
                    ha, hb = 2 * pr, 2 * pr + 1
                    nc.tensor.matmul(oa[:], vt[:, ha * VW:(ha + 1) * VW],
                                     ea[:], start=(kc == 0),
                                     stop=(kc == KC - 1))
                    nc.tensor.matmul(ob[:], vt[:, hb * VW:(hb + 1) * VW],
                                     eb[:], start=(kc == 0),
                                     stop=(kc == KC - 1))

                # normalize rows 0:DK by row DK (the ones-column sums)
                ra = misc.tile([1, QB], F32, tag="ra", name="ra", bufs=1)
                rb = misc.tile([1, QB], F32, tag="rb", name="rb", bufs=1)
                nc.vector.reciprocal(ra[:], oa[DK:VW, :])
                nc.vector.reciprocal(rb[:], ob[DK:VW, :])
                bc = misc.tile([P, QB], F32, tag="bc", name="bc")
                nc.gpsimd.partition_broadcast(bc[0:DK, :], ra[:],
                                              channels=DK)
                nc.sync.dma_start(
                    bc[DK:P, :],
                    rb[:].unsqueeze(1).to_broadcast((1, DK, QB)))
                ot = otp.tile([P, QB], F32R, tag="ot", name="ot")
                nc.vector.tensor_mul(ot[0:DK, :], oa[0:DK, :], bc[0:DK, :])
                nc.vector.tensor_mul(ot[DK:P, :], ob[0:DK, :], bc[DK:P, :])
                ot_tiles.append(ot)

            qsl = bass.ts(qb, QB)
            for n in range(NOC):
                yps = ps_b.tile([P, QB], F32, tag="sp", name="yps")
                for pr in range(NPAIR):
                    nc.tensor.matmul(
                        yps[:],
                        wo_sb[:, pr * D + n * P: pr * D + (n + 1) * P],
                        ot_tiles[pr][:],
                        start=(pr == 0), stop=(pr == NPAIR - 1))
                ysb = misc.tile([P, QB], F32, tag="ysb", name="ysb")
                nc.vector.tensor_scalar_add(ysb[:], yps[:], bo_sb[:, n:n + 1])
                nc.sync.dma_start(yt_dram3[:, n, qsl], ysb[:])


_CACHE = {}


def _get_nc():
    if "nc" not in _CACHE:
        _CACHE["nc"] = build_bass()
    return _CACHE["nc"]


def host_prep(x, Wq, bq, Wk, bk, Wv, bv, Wo, bo):
    """Build the 8 per-core input maps."""
    NPAIR = HL // 2
    NOC = D_MODEL // P
    in_maps = []
    for core in range(N_CORES):
        b, g = divmod(core, 2)
        lo, hi = g * HL * DK, (g + 1) * HL * DK
        in_maps.append({
            "xT": np.ascontiguousarray(x[b].T),
            "Wq": np.ascontiguousarray(Wq[:, lo:hi]),
            "Wk": np.ascontiguousarray(Wk[:, lo:hi]),
            "Wv": np.ascontiguousarray(Wv[:, lo:hi]),
            "Wo": np.ascontiguousarray(Wo[lo:hi, :]),
            "bq_t": np.ascontiguousarray(bq[lo:hi].reshape(NPAIR, P).T),
            "bk_t": np.ascontiguousarray(bk[lo:hi].reshape(NPAIR, P).T),
            "bv_bc": np.broadcast_to(bv[lo:hi], (P, HL * DK)).copy(),
            "bo_t": np.ascontiguousarray((bo * 0.5).reshape(NOC, P).T),
        })
    return in_maps


def host_gather(results):
    """Sum head-group partials and transpose back to [B, S, D]."""
    out = np.empty((BATCH, SEQ, D_MODEL), dtype=np.float32)
    for b in range(BATCH):
        yt = results[2 * b]["YT"] + results[2 * b + 1]["YT"]
        out[b] = yt.T
    return out


def kernel(x, Wq, bq, Wk, bk, Wv, bv, Wo, bo):
    nc = _get_nc()
    in_maps = host_prep(x, Wq, bq, Wk, bk, Wv, bv, Wo, bo)
    res = run_bass_kernel_spmd(nc, in_maps, core_ids=list(range(N_CORES)))
    return host_gather(res.results)
